# revision 7
# baseline (speedup 1.0000x reference)
import numpy as np

import concourse.bacc as bacc
import concourse.bass as bass
import concourse.mybir as mybir
from concourse.bass_utils import run_bass_kernel_spmd
from concourse.tile import TileContext

N_CORES = 8
Q, UNITS, D = 2048, 512, 128
D2 = 2 * D
QS = Q // N_CORES
UT = UNITS // 128

DT_NAME = "fp32"
G = 16
N_GP = 0
USE_TREE = True


def _dt():
    return mybir.dt.float16 if DT_NAME == "fp16" else mybir.dt.float32


def _np_dt():
    return np.float16 if DT_NAME == "fp16" else np.float32


def build_nc():
    dt = _dt()
    f32 = mybir.dt.float32
    nc = bacc.Bacc("TRN2", target_bir_lowering=False)
    x2 = nc.dram_tensor("x2", [QS, D2], dt, kind="ExternalInput")
    w2 = nc.dram_tensor("w2", [UNITS, D2], dt, kind="ExternalInput")
    out = nc.dram_tensor("out", [UT, 128, QS], f32, kind="ExternalOutput")

    n_chunks = QS // G

    with TileContext(nc) as tc:
        with (
            tc.tile_pool(name="wpool", bufs=1) as wpool,
            tc.tile_pool(name="xpool", bufs=2) as xpool,
            tc.tile_pool(name="dpool", bufs=2) as dpool,
            tc.tile_pool(name="opool", bufs=1) as opool,
        ):
            w2sb = wpool.tile([128, UT, D2], dt)
            nc.sync.dma_start(w2sb[:, :, :], w2.rearrange("(t p) d -> p t d", p=128))
            w2c = wpool.tile([128, UT, D2], dt)
            nc.vector.tensor_copy(w2c[:, :, :], w2sb[:, :, :])
            if N_GP > 0:
                w2g = wpool.tile([128, UT, D2], dt)
                nc.gpsimd.tensor_copy(w2g[:, :, :], w2sb[:, :, :])

            osb = opool.tile([128, UT, QS], f32)

            for c in range(n_chunks):
                x2b = xpool.tile([128, G * D2], dt, tag="x2b")
                src = (
                    x2[c * G : (c + 1) * G, :]
                    .rearrange("g d -> (g d)")
                    .unsqueeze(0)
                    .broadcast_to([128, G * D2])
                )
                nc.sync.dma_start(x2b[:, :], src)
                x2b3 = x2b.rearrange("p (g d) -> p g d", d=D2)

                for t in range(UT):
                    on_gp = ((t + c) % UT) < N_GP
                    eng = nc.gpsimd if on_gp else nc.vector
                    wsrc = w2g if on_gp else w2c
                    w2bc = wsrc[:, t : t + 1, :].broadcast_to([128, G, D2])
                    diff = dpool.tile([128, G, D2], dt, tag=f"diff{int(on_gp)}")
                    eng.tensor_tensor(diff[:], x2b3, w2bc, mybir.AluOpType.subtract)

                    cur, width = diff, D2
                    if USE_TREE:
                        min_w = 16 if not on_gp else 1
                        while width > min_w and width % 2 == 0:
                            half = width // 2
                            if on_gp and half == 1:
                                nxt = osb[:, t, c * G : (c + 1) * G].unsqueeze(2)
                            else:
                                nxt = dpool.tile(
                                    [128, G, half], dt, tag=f"tr{int(on_gp)}_{half}"
                                )
                            eng.tensor_tensor(
                                nxt[:, :, :] if hasattr(nxt, "shape") else nxt,
                                cur[:, :, 0:half],
                                cur[:, :, half:width],
                                mybir.AluOpType.min,
                            )
                            cur, width = nxt, half
                    if not on_gp or width > 1:
                        red_eng = nc.vector
                        red_eng.tensor_reduce(
                            osb[:, t, c * G : (c + 1) * G],
                            cur[:, :, :],
                            axis=mybir.AxisListType.X,
                            op=mybir.AluOpType.min,
                        )

            for t in range(UT):
                nc.sync.dma_start(out[t, :, :], osb[:, t, :])

    nc.compile()
    return nc


def _prep_inputs(x, Wmin, Wmax):
    ndt = _np_dt()
    x2 = np.concatenate([x, -x], axis=1).astype(ndt)
    w2 = np.concatenate([Wmin, -Wmax], axis=1).astype(ndt)
    in_maps = []
    for r in range(N_CORES):
        in_maps.append(
            {
                "x2": np.ascontiguousarray(x2[r * QS : (r + 1) * QS]),
                "w2": np.ascontiguousarray(w2),
            }
        )
    return in_maps


def _assemble(results):
    ys = []
    for r in range(N_CORES):
        o = results[r]["out"]
        ys.append(o.reshape(UNITS, QS).T)
    return np.ascontiguousarray(np.concatenate(ys, axis=0).astype(np.float32))


_NC_CACHE = {}


def _get_nc():
    key = (DT_NAME, G, N_GP, USE_TREE)
    if key not in _NC_CACHE:
        _NC_CACHE[key] = build_nc()
    return _NC_CACHE[key]


def run(x, Wmin, Wmax, trace=False):
    nc = _get_nc()
    in_maps = _prep_inputs(x, Wmin, Wmax)
    res = run_bass_kernel_spmd(nc, in_maps, core_ids=list(range(N_CORES)), trace=trace)
    return _assemble(res.results), res


def kernel(x, Wmin, Wmax):
    y, _ = run(x, Wmin, Wmax, trace=False)
    return y
